# revision 11
# baseline (speedup 1.0000x reference)
"""Char-level BiLSTM embedder on 8 NeuronCores (Trainium2, Bass/Tile).

Computation: x[B=32,T=128,L=16] char ids -> embed[E=512] -> fwd+bwd LSTM(H=256)
over the L=16 chars of each of the N=B*T=4096 independent words -> final hidden
states concatenated -> y[B,T,2H=512].

v4 design:
  - Data parallel over N: 512 words per core.
  - fp16 storage for all 2-byte tensors (same PE/DVE speed as bf16, 4x the
    mantissa: rel err 5.9e-4 vs 4.6e-3).
  - Step 0 folded on host: h1/c1 for each word shipped via DMA (char-0 only
    depends on the 128-entry vocab, so host computes 128 states and gathers).
    Device loop runs t=1..15 only: -6% of all engine work.
  - Embedding lookup + input projection + bias fused on host into a
    [V=128, 4H] LUT per direction; per-step input contribution is a K=128
    matmul with one-hot rhs accumulated into the same PSUM group as the
    recurrent h matmuls.
  - Everything device-side is feature-major: gates/c/h live as
    [feature-chunk on partitions, words on free dim].
  - PSUM as two 4-bank tiles per step-dir (i0,i1,f0,f1 | g0,g1,o0,o1),
    one shared tag with bufs=2 so PE fill overlaps ACT drain.
  - Gate order permuted to (i,f,o,g); fwd/bwd interleave per step.
  - DMA priority order: fwd-direction tensors first so compute starts ~3us in.
"""

import sys

sys.path.insert(0, "/opt/trn_rl_repo")

import numpy as np
import concourse.bass as bass
import concourse.bacc as bacc
import concourse.mybir as mybir
import concourse.tile as tile
from concourse.bass_utils import run_bass_kernel_spmd
from concourse.tile_rust import add_dep_helper

# problem constants (hardcoded per harness contract)
B, T, L = 32, 128, 16
VOCAB, E, H = 128, 512, 256
G4 = 4 * H  # 1024
N_CORES = 8
NW = (B * T) // N_CORES  # 512 words per core

F32 = mybir.dt.float32
DT = mybir.dt.float16
NP_DT = np.float16
F8 = mybir.dt.float8e4
H_FP8 = True  # h-recurrence as batched fp8 DoubleRow matmuls
import ml_dtypes
NP_F8 = ml_dtypes.float8_e4m3fn
WDT = F8 if H_FP8 else DT
NP_WDT = NP_F8 if H_FP8 else NP_DT
DRM = mybir.MatmulPerfMode.DoubleRow

AFT = mybir.ActivationFunctionType
ALU = mybir.AluOpType

# tanh(x) ~= x*(PC0 + PC1*x^2 + PC2*x^4) on |x|<=0.9 (|c| <= ~0.4 here);
# used to offload the fwd direction's tanh(c) from ACT to DVE.
PC0, PC1, PC2 = 0.99961288, -0.32273561, 0.08954617


def build_nc():
    nc = bacc.Bacc()

    oh_d = nc.dram_tensor("oh", [L, VOCAB, NW], DT, kind="ExternalInput")
    fused_dd = {
        d: nc.dram_tensor(f"fused_{d}", [VOCAB, G4], DT, kind="ExternalInput")
        for d in "fb"
    }
    whh_dd = {
        d: nc.dram_tensor(f"whh_{d}", [2, 128, G4], WDT, kind="ExternalInput")
        for d in "fb"
    }
    h1_dd = {
        d: nc.dram_tensor(f"h1_{d}", [128, 2 * NW], WDT, kind="ExternalInput")
        for d in "fb"
    }
    c1_dd = {
        d: nc.dram_tensor(f"c1_{d}", [128, 2 * NW], DT, kind="ExternalInput")
        for d in "fb"
    }
    hout_d = nc.dram_tensor("hout", [128, 4 * NW], DT, kind="ExternalOutput")

    with tile.TileContext(nc) as tc:
        with (
            tc.tile_pool(name="const", bufs=1) as cpool,
            tc.tile_pool(name="work", bufs=2) as wpool,
            tc.tile_pool(name="state", bufs=2) as spool,
            tc.tile_pool(name="psum", bufs=2, space=bass.MemorySpace.PSUM) as ppool,
        ):
            # --- load constants, first-needed first -----------------------
            fused = {}
            whh = {}
            whh_w = {}
            h1 = {}
            c1 = {}
            oh_sb = {}

            def load_dir_consts(d):
                # chunked DMAs so the transfers spread across DMA queues and
                # the first compute wave can start a few us in
                fu = cpool.tile([128, G4], DT, name=f"fused_{d}_sb", tag=f"fused_{d}")
                for q in range(2):
                    nc.sync.dma_start(
                        fu[:, q * 512 : (q + 1) * 512],
                        fused_dd[d][:, q * 512 : (q + 1) * 512],
                    )
                fused[d] = fu
                t1 = 1 if d == "f" else L - 2
                ot = cpool.tile([128, NW], DT, name=f"oh_{t1}", tag=f"oh_{t1}")
                nc.sync.dma_start(ot[:], oh_d[t1])
                oh_sb[t1] = ot
                w = cpool.tile([128, 2 * G4], WDT, name=f"whh_{d}_sb", tag=f"whh_{d}")
                for k in range(2):
                    for q in range(2):
                        nc.sync.dma_start(
                            w[:, k * G4 + q * 512 : k * G4 + (q + 1) * 512],
                            whh_dd[d][k, :, q * 512 : (q + 1) * 512],
                        )
                whh[d] = [w[:, 0:G4], w[:, G4 : 2 * G4]]
                whh_w[d] = w
                ht = cpool.tile([128, 2 * NW], WDT, name=f"h1_{d}", tag=f"h1_{d}")
                nc.sync.dma_start(ht[:], h1_dd[d][:])
                h1[d] = ht
                ct = cpool.tile([128, 2 * NW], DT, name=f"c1_{d}", tag=f"c1_{d}")
                nc.sync.dma_start(ct[:], c1_dd[d][:])
                c1[d] = ct

            load_dir_consts("f")
            load_dir_consts("b")
            # bulk one-hots for chars 2..13, then 15 and 0 (needed last)
            for lo, hi in ((2, 8), (8, 14)):
                om = cpool.tile(
                    [128, (hi - lo) * NW], DT, name=f"oh_m{lo}", tag=f"oh_m{lo}"
                )
                nc.sync.dma_start(
                    om[:].rearrange("p (t n) -> p t n", t=hi - lo),
                    oh_d[lo:hi].rearrange("t p n -> p t n"),
                )
                oh_sb[(lo, hi)] = om
            for t in (15, 0):
                ot = cpool.tile([128, NW], DT, name=f"oh_{t}", tag=f"oh_{t}")
                nc.sync.dma_start(ot[:], oh_d[t])
                oh_sb[t] = ot

            def oh_rhs(t):
                v = oh_sb.get(t)
                if v is not None:
                    return v[:]
                lo = 2 if t < 8 else 8
                hi = 8 if t < 8 else 14
                return oh_sb[(lo, hi)][:, (t - lo) * NW : (t - lo + 1) * NW]

            out_sb = cpool.tile([128, 4 * NW], DT, name="out_sb", tag="out_sb")

            # HAM warm-up: dummy matmuls on a zeroed tile, issued while the
            # input DMAs are still in flight, so the PE clock gate reaches
            # 2.4 GHz before the first real matmul.
            warm_src = wpool.tile([128, NW], DT, name="warm_src", tag="warm_src", bufs=1)
            nc.gpsimd.memset(warm_src[:], 0.0)
            warm_ps = ppool.tile([128, 4 * NW], F32, name="warm_ps", tag="ps")
            for wj in range(22):
                nc.tensor.matmul(
                    warm_ps[:, (wj % 4) * NW : (wj % 4) * NW + 128],
                    warm_src[:, 0:128],
                    warm_src[:, 0:128],
                    start=True,
                    stop=True,
                )

            c_cur = {"f": c1["f"], "b": c1["b"]}
            h_cur = {"f": h1["f"], "b": h1["b"]}

            # psum_a chunks: i0,i1,f0,f1 (all sigmoid)
            # psum_b chunks: g0,g1,o0,o1 (tanh first so the cell-update
            #   chain can start while the o matmuls still run)
            B_GC = (6, 7, 4, 5)  # psum_b slice jj -> global gate chunk

            def emit_mms(d, t):
                tchar = t if d == "f" else L - 1 - t
                rhs_oh = oh_rhs(tchar)
                h_prev = h_cur[d]
                psum_a = ppool.tile([128, 4 * NW], F32, name="psum_a", tag="ps")
                psum_b = ppool.tile([128, 4 * NW], F32, name="psum_b", tag="ps")
                # all LUT (one-hot) matmuls first: they depend only on
                # constants, so the PE can run them while h is still being
                # computed; the h matmuls follow.
                for ps, gcs in ((psum_a, (0, 1, 2, 3)), (psum_b, B_GC)):
                    for jj, gc in enumerate(gcs):
                        sl = ps[:, jj * NW : (jj + 1) * NW]
                        lhs_f = fused[d][:, gc * 128 : (gc + 1) * 128]
                        nc.tensor.matmul(sl, lhs_f, rhs_oh, start=True, stop=False)
                if H_FP8:
                    rhs_h2 = h_prev[:].rearrange("p (k n) -> p k n", k=2)
                    w2 = whh_w[d][:].rearrange("p (k g) -> p k g", k=2)
                    for ps, gcs in ((psum_a, (0, 1, 2, 3)), (psum_b, B_GC)):
                        for jj, gc in enumerate(gcs):
                            sl = ps[:, jj * NW : (jj + 1) * NW]
                            nc.tensor.matmul(
                                sl, w2[:, :, gc * 128 : (gc + 1) * 128], rhs_h2,
                                start=False, stop=True, perf_mode=DRM,
                            )
                else:
                    for ps, gcs in ((psum_a, (0, 1, 2, 3)), (psum_b, B_GC)):
                        for jj, gc in enumerate(gcs):
                            sl = ps[:, jj * NW : (jj + 1) * NW]
                            for k in range(2):
                                lhs_h = whh[d][k][:, gc * 128 : (gc + 1) * 128]
                                rhs_h = h_prev[:, k * NW : (k + 1) * NW]
                                nc.tensor.matmul(
                                    sl, lhs_h, rhs_h, start=False, stop=(k == 1)
                                )
                return psum_a, psum_b

            def emit_gates_ifg(d, psum_a, psum_b):
                sig_if = wpool.tile([128, 4 * NW], DT, name="sig_if", tag=f"sig_if_{d}")
                nc.scalar.activation(sig_if[:], psum_a[:], AFT.Sigmoid)
                tanh_g = wpool.tile([128, 2 * NW], DT, name="tanh_g", tag=f"tanh_g_{d}")
                i_tg = nc.scalar.activation(tanh_g[:], psum_b[:, 0 : 2 * NW], AFT.Tanh)
                return sig_if, tanh_g, i_tg

            def emit_sig_o(d, psum_b):
                sig_o = wpool.tile([128, 2 * NW], DT, name="sig_o", tag=f"sig_o_{d}")
                nc.scalar.activation(sig_o[:], psum_b[:, 2 * NW : 4 * NW], AFT.Sigmoid)
                return sig_o

            def emit_cell(d, t, sig_if, tanh_g):
                # c = sig(f) * c + sig(i) * tanh(g)
                c_prev = c_cur[d]
                c_new = spool.tile([128, 2 * NW], DT, name=f"c_{d}", tag=f"c_{d}")
                m1 = wpool.tile([128, 2 * NW], DT, name="m1", tag=f"m1_{d}")
                nc.vector.tensor_mul(m1[:], sig_if[:, 2 * NW : 4 * NW], c_prev[:])
                m2 = wpool.tile([128, 2 * NW], DT, name="m2", tag=f"m2_{d}")
                nc.vector.tensor_mul(m2[:], sig_if[:, 0 : 2 * NW], tanh_g[:])
                nc.vector.tensor_add(c_new[:], m1[:], m2[:])
                c_cur[d] = c_new

            def emit_tanh_c(d, c_tile, after=None):
                tanh_c = wpool.tile([128, 2 * NW], DT, name="tanh_c", tag=f"tanh_c_{d}")
                i = nc.scalar.activation(tanh_c[:], c_tile[:], AFT.Tanh)
                if after is not None:
                    # scheduler ordering only (see v1): order this activation
                    # relative to the other direction's gate activations
                    add_dep_helper(after.ins, i.ins, sync=False, reason="act order")
                return tanh_c

            def emit_tanh_c_poly(d, c_tile):
                # tanh(c) ~= c*(PC0 + PC1*u + PC2*u^2), u = c^2, on DVE --
                # offloads the fwd tanh_c from the bottleneck ACT engine
                u = wpool.tile([128, 2 * NW], DT, name="u", tag=f"u_{d}")
                nc.vector.tensor_mul(u[:], c_tile[:], c_tile[:])
                r = wpool.tile([128, 2 * NW], DT, name="r", tag=f"r_{d}")
                nc.vector.tensor_scalar(
                    out=r[:], in0=u[:], scalar1=PC2, scalar2=PC1,
                    op0=ALU.mult, op1=ALU.add,
                )
                r2 = wpool.tile([128, 2 * NW], DT, name="r2", tag=f"r2_{d}")
                nc.vector.tensor_mul(r2[:], r[:], u[:])
                tanh_c = wpool.tile([128, 2 * NW], DT, name="tanh_cp", tag=f"tanh_c_{d}")
                nc.vector.scalar_tensor_tensor(
                    tanh_c[:], r2[:], PC0, c_tile[:], ALU.add, ALU.mult
                )
                return tanh_c

            def emit_h(d, t, sig_o, tanh_c):
                # h = sig(o) * tanh(c)
                if t == L - 1:
                    off = 0 if d == "f" else 2 * NW
                    nc.vector.tensor_mul(
                        out_sb[:, off : off + 2 * NW], sig_o[:], tanh_c[:]
                    )
                    nc.sync.dma_start(
                        hout_d[:, off : off + 2 * NW], out_sb[:, off : off + 2 * NW]
                    )
                else:
                    h_new = spool.tile([128, 2 * NW], WDT, name=f"h_{d}", tag=f"h_{d}")
                    nc.vector.tensor_mul(h_new[:], sig_o[:], tanh_c[:])
                    h_cur[d] = h_new

            pending_b = None  # (t, sig_o_b, c_tile) awaiting next step's gates
            for t in range(1, L):
                pa_f, pb_f = emit_mms("f", t)
                sig_if_f, tanh_g_f, i_tg_f = emit_gates_ifg("f", pa_f, pb_f)
                if pending_b is not None:
                    pt, p_sig_o, p_c = pending_b
                    tc_pb = emit_tanh_c("b", p_c, after=i_tg_f)
                    emit_h("b", pt, p_sig_o, tc_pb)
                pa_b, pb_b = emit_mms("b", t)
                emit_cell("f", t, sig_if_f, tanh_g_f)
                sig_o_f = emit_sig_o("f", pb_f)
                sig_if_b, tanh_g_b, i_tg_b = emit_gates_ifg("b", pa_b, pb_b)
                tc_f = emit_tanh_c("f", c_cur["f"], after=i_tg_b)
                emit_cell("b", t, sig_if_b, tanh_g_b)
                sig_o_b = emit_sig_o("b", pb_b)
                emit_h("f", t, sig_o_f, tc_f)
                pending_b = (t, sig_o_b, c_cur["b"])
            pt, p_sig_o, p_c = pending_b
            tc_pb = emit_tanh_c("b", p_c)
            emit_h("b", pt, p_sig_o, tc_pb)

    nc.compile()
    return nc


_NC_CACHE = None


def _get_nc():
    global _NC_CACHE
    if _NC_CACHE is None:
        _NC_CACHE = build_nc()
    return _NC_CACHE


# gate permutation: torch order (i,f,g,o) -> device order (i,f,o,g)
_PERM = np.concatenate([np.arange(0, 512), np.arange(768, 1024), np.arange(512, 768)])


def prepare_in_maps(x, embed_table, w_ih_f, w_hh_f, b_ih_f, b_hh_f,
                    w_ih_b, w_hh_b, b_ih_b, b_hh_b):
    ids = np.asarray(x).reshape(B * T, L).astype(np.int64)

    shared = {}
    step0 = {}
    for d, w_ih, w_hh, b_ih, b_hh in (
        ("f", w_ih_f, w_hh_f, b_ih_f, b_hh_f),
        ("b", w_ih_b, w_hh_b, b_ih_b, b_hh_b),
    ):
        w_ih = np.asarray(w_ih, np.float32)[_PERM]
        w_hh = np.asarray(w_hh, np.float32)[_PERM]
        bias = (np.asarray(b_ih, np.float32) + np.asarray(b_hh, np.float32))[_PERM]
        fused = np.asarray(embed_table, np.float32) @ w_ih.T + bias[None, :]
        shared[f"fused_{d}"] = np.ascontiguousarray(fused.astype(NP_DT))
        shared[f"whh_{d}"] = np.ascontiguousarray(
            w_hh.T.reshape(2, 128, G4).astype(NP_WDT)
        )
        # step-0 tables per char: h1[c], c1[c] (host fp64 math)
        g = fused.astype(np.float64)  # [V, 4H] in (i,f,o,g) chunk order
        i, f, o, gg = g[:, 0:256], g[:, 256:512], g[:, 512:768], g[:, 768:1024]
        sg = lambda v: 1.0 / (1.0 + np.exp(-v))
        c1t = sg(i) * np.tanh(gg)  # [V, 256]
        h1t = sg(o) * np.tanh(c1t)
        step0[d] = (h1t, c1t)

    vrange = np.arange(VOCAB)
    in_maps = []
    for cix in range(N_CORES):
        ids_c = ids[cix * NW : (cix + 1) * NW]  # [NW, L]
        oh = (ids_c.T[:, None, :] == vrange[None, :, None]).astype(NP_DT)  # [L,V,NW]
        m = dict(shared)
        m["oh"] = np.ascontiguousarray(oh)
        for d in "fb":
            h1t, c1t = step0[d]
            ch0 = ids_c[:, 0] if d == "f" else ids_c[:, L - 1]
            m[f"h1_{d}"] = np.ascontiguousarray(
                h1t[ch0].T.reshape(2, 128, NW).transpose(1, 0, 2)
                .reshape(128, 2 * NW).astype(NP_WDT)
            )
            m[f"c1_{d}"] = np.ascontiguousarray(
                c1t[ch0].T.reshape(2, 128, NW).transpose(1, 0, 2)
                .reshape(128, 2 * NW).astype(NP_DT)
            )
        in_maps.append(m)
    return in_maps


def assemble_output(results):
    ys = []
    for c in range(N_CORES):
        hout = results[c]["hout"].astype(np.float32)  # [128, 4*NW]
        hf = np.concatenate([hout[:, 0:NW], hout[:, NW : 2 * NW]], axis=0)  # [H,NW]
        hb = np.concatenate([hout[:, 2 * NW : 3 * NW], hout[:, 3 * NW : 4 * NW]], axis=0)
        ys.append(np.concatenate([hf.T, hb.T], axis=1))  # [NW, 2H]
    y = np.concatenate(ys, axis=0)  # [B*T, 2H]
    return y.reshape(B, T, 2 * H)


def run(in_maps, trace=False):
    nc = _get_nc()
    res = run_bass_kernel_spmd(nc, in_maps, core_ids=list(range(N_CORES)), trace=trace)
    return res


def kernel(**inputs) -> np.ndarray:
    in_maps = prepare_in_maps(**inputs)
    res = run(in_maps, trace=False)
    return assemble_output(res.results)
